# revision 48
# baseline (speedup 1.0000x reference)
"""Trainium2 Bass kernel for nn_DecoderLayer_19816979104174.

Data-parallel over batch: each of the 8 NeuronCores runs one batch element's
full decoder layer. v3 design:
  - Host-folded weights: A_h = Wq_h @ Wk_h^T collapses the Q and K projections
    into one (scores = x A x^T); Wvo_h = Wv_h @ Wo_h collapses the V projection
    and the output projection (bf16); Wom_h (cross) is applied to mem_values in
    fp8 DoubleRow. Bias terms that are not softmax-invariant fold into the
    per-s exp bias column (r_self / r_mem, host-computed).
  - fp8 e4m3 DoubleRow matmuls (2x PE rate) for the score path, the
    attention-weighted sums, the cross value fold, and the FFN; bf16 for the
    self value fold only (accuracy).
  - exp writes fp8 directly; rowsums via 128-wide fp8 ones matmuls (result
    lands pre-broadcast in PSUM); softmax reciprocals and LN rsqrt run on the
    scalar engine; attention weights are normalized in place so the AV matmuls
    accumulate all 8 heads directly in PSUM in [t, d] layout, consumed by
    LayerNorm straight out of PSUM.
  - x / memk / memv arrive pre-transposed (and pre-quantized) from the host;
    weights arrive pre-shuffled into partition-contiguous layout; input DMAs
    are split across two hardware queues (sync: x + weight streams,
    scalar/Activation: mem + residual tensors).
"""

import sys

sys.path.insert(0, "/opt/trn_rl_repo")
sys.path.insert(0, "/root/.axon_site/_ro/trn_rl_repo")

import numpy as np

B, T, S, D, H, F = 8, 1024, 1024, 512, 8, 2048
P = 128
ND, NS, NT, NF, NC2 = D // P, S // P, T // P, F // P, T // 512
SCALE = 1.0 / float(np.sqrt(D))
LN_EPS = 1e-5

# fp8 scale constants (compile-time)
SX = 16.0       # x, x1, x2, memk, memv quantization scale
SXA = 64.0      # xA (score-path projection) fp8 scale
SQM = 32.0      # qm fp8 scale
SV = 64.0       # v' / mv fp8 scale
SNORM = 128.0   # normalized attention weights are stored as a*SNORM
F1S = 64.0      # f1 (relu output) fp8 scale
CSA = 4096.0    # host scale on A_h
CSQ = 1536.0    # host scale on Wqm_h
CSOM = 1536.0   # host scale on Wo_mem (fp8)
CSW1 = 1536.0   # host scale on W1
CSW2 = 1536.0   # host scale on W2
ES_SELF = 32.0  # exp output scale, self attention
ES_CROSS = 8.0  # exp output scale, cross attention

_CACHE = {}


def _build():
    if "nc" in _CACHE:
        return _CACHE["nc"]

    import concourse.tile as tile
    import concourse.mybir as mybir
    from concourse import bacc
    from concourse.masks import make_identity
    from contextlib import ExitStack

    bf16 = mybir.dt.bfloat16
    f32 = mybir.dt.float32
    fp8 = mybir.dt.float8e4
    AF = mybir.ActivationFunctionType
    OP = mybir.AluOpType
    DR = mybir.MatmulPerfMode.DoubleRow

    nc = bacc.Bacc("TRN2")

    # ---- DRAM I/O -----------------------------------------------------
    d_xT8 = nc.dram_tensor("xT8", [P, ND, T], fp8, kind="ExternalInput")
    d_xTb = nc.dram_tensor("xTb", [P, ND, T], bf16, kind="ExternalInput")
    d_x32 = nc.dram_tensor("x32", [P, NT, D], f32, kind="ExternalInput")
    d_mkT8 = nc.dram_tensor("mkT8", [P, ND, S], fp8, kind="ExternalInput")
    d_mv8 = nc.dram_tensor("mv8", [P, ND, S], fp8, kind="ExternalInput")
    d_a8 = nc.dram_tensor("a8", [P, H, ND, ND, P], fp8, kind="ExternalInput")
    d_wvo = nc.dram_tensor("wvo", [P, H, ND, D], bf16, kind="ExternalInput")
    d_wqm8 = nc.dram_tensor("wqm8", [P, H, ND, ND, P], fp8,
                            kind="ExternalInput")
    d_wom8 = nc.dram_tensor("wom8", [P, H, ND, D], fp8, kind="ExternalInput")
    d_w18 = nc.dram_tensor("w18", [P, ND, NF, P], fp8, kind="ExternalInput")
    d_w28 = nc.dram_tensor("w28", [P, NF, D], fp8, kind="ExternalInput")
    d_bs = nc.dram_tensor("bias_self", [P, H * NS], f32, kind="ExternalInput")
    d_bc = nc.dram_tensor("bias_cross", [P, H * NS], f32, kind="ExternalInput")
    d_b1 = nc.dram_tensor("b1_c", [P, NF], f32, kind="ExternalInput")
    d_b2 = nc.dram_tensor("b2_row", [1, D], bf16, kind="ExternalInput")
    d_bom = nc.dram_tensor("bom_row", [1, D], bf16, kind="ExternalInput")
    d_diag = nc.dram_tensor("diag", [P, P], f32, kind="ExternalInput")
    d_out = nc.dram_tensor("out", [T, D], f32, kind="ExternalOutput")

    with tile.TileContext(nc) as tc, ExitStack() as ctx:
        const = ctx.enter_context(tc.tile_pool(name="const", bufs=1))
        small = ctx.enter_context(tc.tile_pool(name="small", bufs=2))
        psum_mm = ctx.enter_context(tc.tile_pool(name="psum_mm", bufs=3, space="PSUM"))
        psum_av = ctx.enter_context(tc.tile_pool(name="psum_av", bufs=2, space="PSUM"))
        psum_rs = ctx.enter_context(tc.tile_pool(name="psum_rs", bufs=2, space="PSUM"))
        psum_tr = ctx.enter_context(tc.tile_pool(name="psum_tr", bufs=1, space="PSUM"))

        # ---- pool opens (lifetime order; DMAs issued below) -----------
        es_big = ExitStack()
        big = es_big.enter_context(tc.tile_pool(name="big", bufs=1))
        expTn = [big.tile([P, NS, T], fp8, tag=f"expTn{h}", name=f"expTn{h}")
                 for h in range(H)]
        v8 = [big.tile([P, NS, D], fp8, tag=f"v8_{h}", name=f"v8_{h}")
              for h in range(H)]

        es_mem = ExitStack()
        mem_pool = es_mem.enter_context(tc.tile_pool(name="mem", bufs=1))
        x1_sb = mem_pool.tile([P, NT, D], f32, tag="x1")
        x1T8 = mem_pool.tile([P, ND, T], fp8, tag="x1T8")
        # per-(head, t-block) softmax inverse row sums, [t_part, 1] layout
        inv1_sb = mem_pool.tile([P, H * NT], f32, tag="inv1")
        inv2_sb = mem_pool.tile([P, H * NT], f32, tag="inv2")
        bom_bc = mem_pool.tile([P, D], f32, tag="bom_bc")

        # mem/residual tensors (scalar hwdge queue) live only through
        # phase 2/3; their pool closes with ws2_stack so the FFN pool can
        # reuse the space
        ws2_stack = ExitStack()
        mem2_pool = ws2_stack.enter_context(tc.tile_pool(name="mem2", bufs=1))
        mkT8 = mem2_pool.tile([P, ND, S], fp8)
        mv8 = mem2_pool.tile([P, ND, S], fp8)
        x32_sb = mem2_pool.tile([P, NT, D], f32)
        wstream2 = ws2_stack.enter_context(tc.tile_pool(name="wstream2", bufs=2))

        es_x = ExitStack()
        x_pool = es_x.enter_context(tc.tile_pool(name="xp", bufs=1))
        ws_stack = ExitStack()
        wstream = ws_stack.enter_context(tc.tile_pool(name="wstream", bufs=2))

        # ---- x + phase-1 weight stream: sync queue issued FIRST so the
        # PE's first matmul inputs land as early as possible ------------
        xT8 = x_pool.tile([P, ND, T], fp8)
        nc.sync.dma_start(out=xT8[:], in_=d_xT8.ap())
        a_tiles = []
        wvo_tiles = []
        xTb = None
        for h in range(H):
            a_t = wstream.tile([P, ND, ND, P], fp8, tag="a8")
            nc.sync.dma_start(out=a_t[:], in_=d_a8.ap()[:, h])
            a_tiles.append(a_t)
            if h == 0:
                xTb = x_pool.tile([P, ND, T], bf16)
                nc.sync.dma_start(out=xTb[:], in_=d_xTb.ap())
            wvo_t = wstream.tile([P, ND, D], bf16, tag="wvo")
            nc.sync.dma_start(out=wvo_t[:], in_=d_wvo.ap()[:, h])
            wvo_tiles.append(wvo_t)

        # ---- constants (gpsimd queue) ---------------------------------
        ident_f = const.tile([P, P], f32)
        make_identity(nc, ident_f)
        ones8 = const.tile([P, 2, P], fp8)
        nc.vector.memset(ones8[:], 1.0 / SNORM)
        ones_row = const.tile([1, P], bf16)
        nc.vector.memset(ones_row[:], 1.0)
        eps_t = const.tile([P, 1], f32)
        nc.vector.memset(eps_t[:], LN_EPS)
        e0c = const.tile([P, 1], bf16)
        nc.vector.memset(e0c[:], 0.0)
        nc.vector.memset(e0c[0:1, :], SNORM * SV)
        diag_sb = const.tile([P, P], f32)
        nc.gpsimd.dma_start(out=diag_sb[:], in_=d_diag.ap())
        bs_sb = const.tile([P, H * NS], f32)
        nc.gpsimd.dma_start(out=bs_sb[:], in_=d_bs.ap())
        bc_sb = const.tile([P, H * NS], f32)
        nc.gpsimd.dma_start(out=bc_sb[:], in_=d_bc.ap())
        b1_sb = const.tile([P, NF], f32)
        nc.gpsimd.dma_start(out=b1_sb[:], in_=d_b1.ap())
        b2_sb = const.tile([1, D], bf16)
        nc.gpsimd.dma_start(out=b2_sb[:], in_=d_b2.ap())
        bom_sb = const.tile([1, D], bf16)
        nc.gpsimd.dma_start(out=bom_sb[:], in_=d_bom.ap())

        # big-tensor zero fills (gpsimd)
        for h in range(H):
            nc.gpsimd.memset(expTn[h][:], 0.0)

        # mem tensors + residual x: scalar (Activation) hwdge queue
        nc.scalar.dma_start(out=mkT8[:], in_=d_mkT8.ap())
        nc.scalar.dma_start(out=mv8[:], in_=d_mv8.ap())
        nc.scalar.dma_start(out=x32_sb[:], in_=d_x32.ap())

        # ---- helpers ---------------------------------------------------
        def ln_psum(ps_ap, inv, resid_ap, out_ap):
            """out = LN(ps*inv + resid)   (gamma=1, beta=0)"""
            res = small.tile([P, D], f32, tag="ln_res")
            nc.vector.scalar_tensor_tensor(
                out=res[:], in0=ps_ap, scalar=inv, in1=resid_ap,
                op0=OP.mult, op1=OP.add)
            stats = small.tile([P, 6], f32, tag="ln_stats")
            nc.vector.bn_stats(stats[:], res[:])
            mv2 = small.tile([P, 2], f32, tag="ln_mv")
            nc.vector.bn_aggr(mv2[:], stats[:])
            std = small.tile([P, 1], f32, tag="ln_std")
            nc.scalar.activation(std[:], mv2[:, 1:2], AF.Sqrt, bias=eps_t[:])
            istd = small.tile([P, 1], f32, tag="ln_istd")
            nc.vector.reciprocal(istd[:], std[:])
            nc.vector.tensor_scalar(
                out=out_ap, in0=res[:], scalar1=mv2[:, 0:1], scalar2=istd[:],
                op0=OP.subtract, op1=OP.mult)

        def ln_sbuf(acc, out_ap):
            """out = LN(acc)   (gamma=1, beta=0); acc is SBUF f32 [P, D]."""
            stats = small.tile([P, 6], f32, tag="ln_stats")
            nc.vector.bn_stats(stats[:], acc[:])
            mv2 = small.tile([P, 2], f32, tag="ln_mv")
            nc.vector.bn_aggr(mv2[:], stats[:])
            std = small.tile([P, 1], f32, tag="ln_std")
            nc.scalar.activation(std[:], mv2[:, 1:2], AF.Sqrt, bias=eps_t[:])
            istd = small.tile([P, 1], f32, tag="ln_istd")
            nc.vector.reciprocal(istd[:], std[:])
            nc.vector.tensor_scalar(
                out=out_ap, in0=acc[:], scalar1=mv2[:, 0:1], scalar2=istd[:],
                op0=OP.subtract, op1=OP.mult)

        def attn_head(h, qsrcT, kT, bias_sb, causal, exp_scale, rsb_pool,
                      inv_sb, filler=None):
            """scores -> exp(fp8, unnormalized) -> rowsum -> inv[t,1], head h.

            qsrcT: fp8 [P, ND, T] query-side (already scaled); kT: fp8 key-side
            [P, ND, S]. Writes raw exp weights into expTn[h] and per-t-block
            reciprocal row sums (x 1/(SNORM*SV)) into inv_sb[:, h*NT + tb].
            filler(c) emits independent PE work between the score matmuls and
            the rowsum (which must wait for the scalar-engine exps).
            """
            for c in range(NC2):
                jmax = 4 * (c + 1) if causal else NS
                for j in range(jmax):
                    lo = max(j * P, c * 512) if causal else c * 512
                    w = (c + 1) * 512 - lo
                    ps = psum_mm.tile([P, 512], f32, tag="mm")
                    for ep in range(2):
                        nc.tensor.matmul(
                            ps[:, :w],
                            lhsT=kT[:, 2 * ep:2 * ep + 2, j * P:(j + 1) * P],
                            rhs=qsrcT[:, 2 * ep:2 * ep + 2, lo:(c + 1) * 512],
                            start=(ep == 0), stop=(ep == 1), perf_mode=DR)
                    if causal and lo == j * P:
                        nc.vector.tensor_tensor(
                            out=ps[:, 0:P], in0=ps[:, 0:P], in1=diag_sb[:],
                            op=OP.add)
                    nc.scalar.activation(
                        expTn[h][:, j, lo:(c + 1) * 512], ps[:, :w], AF.Exp,
                        bias=bias_sb[:, h * NS + j:h * NS + j + 1],
                        scale=exp_scale)
                if filler is not None:
                    filler(c)
                rs = psum_rs.tile([P, 512], f32, tag="rs")
                for jp in range(jmax // 2):
                    nc.tensor.matmul(
                        rs[:], lhsT=ones8[:],
                        rhs=expTn[h][:, 2 * jp:2 * jp + 2, c * 512:(c + 1) * 512],
                        start=(jp == 0), stop=(jp == jmax // 2 - 1), perf_mode=DR)
                # transpose the (partition-broadcast) row sums into [t, 1]
                # via one-hot matmuls, then a tiny reciprocal
                rs_sb = rsb_pool.tile([P, 512], bf16, tag="rssb")
                nc.vector.tensor_scalar_mul(rs_sb[:], rs[:], 1.0)
                for tbi in range(4):
                    nc.tensor.matmul(
                        rs[:, 508 + tbi:509 + tbi],
                        lhsT=rs_sb[:, tbi * P:(tbi + 1) * P], rhs=e0c[:],
                        start=True, stop=True)
                nc.vector.reciprocal_approx_fast(
                    out=inv_sb[:, h * NT + 4 * c:h * NT + 4 * c + 4],
                    in_=rs[:, 508:512])

        # ============ phase 1: self attention scores/weights ============
        with tc.tile_pool(name="xap", bufs=2) as xa_pool, \
             tc.tile_pool(name="rbp", bufs=2) as rsb_pool:
            for h in range(H):
                a_t = a_tiles[h]
                wvo_t = wvo_tiles[h]
                xAT = xa_pool.tile([P, ND, T], fp8, tag="xAT")
                for eb in range(ND):
                    for c in range(NC2):
                        ps = psum_mm.tile([P, 512], f32, tag="mm")
                        for kp in range(2):
                            nc.tensor.matmul(
                                ps[:], lhsT=a_t[:, 2 * kp:2 * kp + 2, eb, :],
                                rhs=xT8[:, 2 * kp:2 * kp + 2, c * 512:(c + 1) * 512],
                                start=(kp == 0), stop=(kp == 1), perf_mode=DR)
                        nc.scalar.activation(
                            xAT[:, eb, c * 512:(c + 1) * 512], ps[:], AF.Copy,
                            scale=SXA / (CSA * SX))

                def vo_fill(c, wvo_t=wvo_t, hh=h):
                    for sb_ in range(4 * c, 4 * c + 4):
                        ps = psum_mm.tile([P, 512], f32, tag="mm")
                        for kt in range(ND):
                            nc.tensor.matmul(
                                ps[:], lhsT=xTb[:, kt, sb_ * P:(sb_ + 1) * P],
                                rhs=wvo_t[:, kt, :],
                                start=(kt == 0), stop=(kt == ND - 1))
                        nc.vector.tensor_scalar_mul(
                            v8[hh][:, sb_, :], ps[:], SV)

                attn_head(h, xAT, xT8, bs_sb, True, SCALE / (SX * SXA),
                          rsb_pool, inv1_sb, filler=vo_fill)
        ws_stack.close()
        es_x.close()

        # phase-3 weight stream: issue all DMAs now (sync queue) so they
        # land during phase 2 (double-buffering emerges from tile WAR deps)
        wqm_tiles = []
        wom_tiles = []
        for h in range(H):
            wqm_t = wstream2.tile([P, ND, ND, P], fp8, tag="wqm")
            nc.sync.dma_start(out=wqm_t[:], in_=d_wqm8.ap()[:, h])
            wqm_tiles.append(wqm_t)
            wom_t = wstream2.tile([P, ND, D], fp8, tag="wom")
            nc.sync.dma_start(out=wom_t[:], in_=d_wom8.ap()[:, h])
            wom_tiles.append(wom_t)

        # ===== phase 2: self AV (per-head PSUM + inv-scaled combine) ====
        for tb in range(NT):
            acc = small.tile([P, D], f32, tag="acc")
            npair = (tb + 2) // 2
            for h in range(H):
                ps_av = psum_av.tile([P, 512], f32, tag="av")
                for jp in range(npair):
                    nc.tensor.matmul(
                        ps_av[:],
                        lhsT=expTn[h][:, 2 * jp:2 * jp + 2, tb * P:(tb + 1) * P],
                        rhs=v8[h][:, 2 * jp:2 * jp + 2, :],
                        start=(jp == 0), stop=(jp == npair - 1),
                        perf_mode=DR)
                nc.vector.scalar_tensor_tensor(
                    out=acc[:], in0=ps_av[:],
                    scalar=inv1_sb[:, h * NT + tb:h * NT + tb + 1],
                    in1=(x32_sb[:, tb, :] if h == 0 else acc[:]),
                    op0=OP.mult, op1=OP.add)
            ln_sbuf(acc, x1_sb[:, tb, :])
            tr = psum_tr.tile([P, 512], f32, tag="tr")
            for dt in range(ND):
                nc.tensor.transpose(
                    tr[:, dt * P:(dt + 1) * P],
                    x1_sb[:, tb, dt * P:(dt + 1) * P], ident_f[:])
            nc.scalar.activation(
                x1T8[:, :, tb * P:(tb + 1) * P],
                tr[:].rearrange("p (d t) -> p d t", d=ND), AF.Copy, scale=SX)

        # ============ phase 3: cross attention scores/weights ===========
        with tc.tile_pool(name="qmp", bufs=2) as qm_pool, \
             tc.tile_pool(name="rbp2", bufs=2) as rsb_pool2:
            for h in range(H):
                wqm_t = wqm_tiles[h]
                wom_t = wom_tiles[h]
                qmT = qm_pool.tile([P, ND, T], fp8, tag="qmT")
                for eb in range(ND):
                    for c in range(NC2):
                        ps = psum_mm.tile([P, 512], f32, tag="mm")
                        for kp in range(2):
                            nc.tensor.matmul(
                                ps[:], lhsT=wqm_t[:, 2 * kp:2 * kp + 2, eb, :],
                                rhs=x1T8[:, 2 * kp:2 * kp + 2, c * 512:(c + 1) * 512],
                                start=(kp == 0), stop=(kp == 1), perf_mode=DR)
                        nc.scalar.activation(
                            qmT[:, eb, c * 512:(c + 1) * 512], ps[:], AF.Copy,
                            scale=SQM / (CSQ * SX))

                def om_fill(c, wom_t=wom_t, hh=h):
                    for sb_ in range(4 * c, 4 * c + 4):
                        ps = psum_mm.tile([P, 512], f32, tag="mm")
                        for kp in range(2):
                            nc.tensor.matmul(
                                ps[:],
                                lhsT=mv8[:, 2 * kp:2 * kp + 2, sb_ * P:(sb_ + 1) * P],
                                rhs=wom_t[:, 2 * kp:2 * kp + 2, :],
                                start=(kp == 0), stop=(kp == 1), perf_mode=DR)
                        nc.vector.tensor_scalar_mul(
                            v8[hh][:, sb_, :], ps[:], SV / (SX * CSOM))

                attn_head(h, qmT, mkT8, bc_sb, False, SCALE / (SX * SQM),
                          rsb_pool2, inv2_sb, filler=om_fill)
        ws2_stack.close()

        es_x2 = ExitStack()
        x2_pool = es_x2.enter_context(tc.tile_pool(name="x2p", bufs=1))
        x2_sb = x2_pool.tile([P, NT, D], f32, tag="x2")
        x2T8 = x2_pool.tile([P, ND, T], fp8, tag="x2T8")

        # FFN weights: DMA now so they land during cross AV
        es_ffn = ExitStack()
        ffn_pool = es_ffn.enter_context(tc.tile_pool(name="ffn", bufs=1))
        w18_t = ffn_pool.tile([P, ND, NF, P], fp8, tag="w18")
        nc.sync.dma_start(out=w18_t[:], in_=d_w18.ap())
        w28_t = ffn_pool.tile([P, NF, D], fp8, tag="w28")
        nc.sync.dma_start(out=w28_t[:], in_=d_w28.ap())
        f1T8 = ffn_pool.tile([P, NF, T], fp8, tag="f1T8")

        def f1_chunk(c):
            for fb in range(NF):
                ps = psum_mm.tile([P, 512], f32, tag="mm")
                for kp in range(2):
                    nc.tensor.matmul(
                        ps[:], lhsT=w18_t[:, 2 * kp:2 * kp + 2, fb, :],
                        rhs=x2T8[:, 2 * kp:2 * kp + 2, c * 512:(c + 1) * 512],
                        start=(kp == 0), stop=(kp == 1), perf_mode=DR)
                nc.scalar.activation(
                    f1T8[:, fb, c * 512:(c + 1) * 512], ps[:], AF.Relu,
                    bias=b1_sb[:, fb:fb + 1], scale=F1S / (SX * CSW1))

        # materialize the bom bias row broadcast across partitions (once)
        ps_bb = psum_av.tile([P, 512], f32, tag="av")
        nc.tensor.matmul(
            ps_bb[:], lhsT=ones_row[:, 0:P], rhs=bom_sb[:],
            start=True, stop=True)
        nc.vector.tensor_scalar_mul(bom_bc[:], ps_bb[:], 1.0)

        def f2_block(tb):
            ps = psum_mm.tile([P, 512], f32, tag="mm")
            nc.tensor.matmul(
                ps[:], lhsT=ones_row[:, 0:P], rhs=b2_sb[:],
                start=True, stop=False)
            for kp in range(NF // 2):
                nc.tensor.matmul(
                    ps[:], lhsT=f1T8[:, 2 * kp:2 * kp + 2, tb * P:(tb + 1) * P],
                    rhs=w28_t[:, 2 * kp:2 * kp + 2, :],
                    start=False, stop=(kp == NF // 2 - 1), perf_mode=DR)
            out_sb = small.tile([P, D], f32, tag="out_sb")
            ln_psum(ps[:], 1.0 / (F1S * CSW2), x2_sb[:, tb, :], out_sb[:])
            nc.sync.dma_start(
                out=d_out.ap().rearrange("(tb p) d -> p tb d", p=P)[:, tb, :],
                in_=out_sb[:])

        def x2_transpose(tb):
            tr = psum_tr.tile([P, 512], f32, tag="tr")
            for dt in range(ND):
                nc.tensor.transpose(
                    tr[:, dt * P:(dt + 1) * P],
                    x2_sb[:, tb, dt * P:(dt + 1) * P], ident_f[:])
            nc.scalar.activation(
                x2T8[:, :, tb * P:(tb + 1) * P],
                tr[:].rearrange("p (d t) -> p d t", d=ND), AF.Copy,
                scale=SX)

        # ===== phase 4: cross AV (per-head PSUM + combine) + LN2 + FFN ==
        for tb in range(NT):
            acc = small.tile([P, D], f32, tag="acc")
            for h in range(H):
                ps_av = psum_av.tile([P, 512], f32, tag="av")
                for jp in range(NS // 2):
                    nc.tensor.matmul(
                        ps_av[:],
                        lhsT=expTn[h][:, 2 * jp:2 * jp + 2, tb * P:(tb + 1) * P],
                        rhs=v8[h][:, 2 * jp:2 * jp + 2, :],
                        start=(jp == 0), stop=(jp == NS // 2 - 1),
                        perf_mode=DR)
                nc.vector.scalar_tensor_tensor(
                    out=acc[:], in0=ps_av[:],
                    scalar=inv2_sb[:, h * NT + tb:h * NT + tb + 1],
                    in1=(x1_sb[:, tb, :] if h == 0 else acc[:]),
                    op0=OP.mult, op1=OP.add)
            nc.gpsimd.tensor_tensor(
                out=acc[:], in0=acc[:], in1=bom_bc[:], op=OP.add)
            ln_sbuf(acc, x2_sb[:, tb, :])
            if tb < 4:
                x2_transpose(tb)
            if tb == 3:
                f1_chunk(0)
            if tb == 6:
                # before tb7's (bank-limited) AV chain, emit ready filler:
                # chunk-1 transposes for tb4/5 and the first two f2 blocks
                for ftb in (4, 5):
                    x2_transpose(ftb)
                    f2_block(ftb - 4)

        # remaining tail: f2 for t-blocks 0-3 only needs f1 chunk 0; the
        # tb7 transpose waits on its LayerNorm chain, so ready f2 blocks
        # run first
        x2_transpose(6)
        f2_block(2)
        f2_block(3)
        x2_transpose(7)
        f1_chunk(1)
        for tb in range(4, NT):
            f2_block(tb)
        es_ffn.close()
        es_x2.close()
        es_mem.close()
        es_big.close()

    nc.compile()
    _CACHE["nc"] = nc
    return nc


def make_in_maps(inputs):
    import ml_dtypes

    bf = ml_dtypes.bfloat16
    e4 = ml_dtypes.float8_e4m3
    f32 = np.float32

    def q8(a, s):
        return np.clip(np.asarray(a, f32) * s, -240.0, 240.0).astype(e4)

    x = np.asarray(inputs["x"], f32)
    memk = np.asarray(inputs["mem_keys"], f32)
    memv = np.asarray(inputs["mem_values"], f32)
    Wq = np.asarray(inputs["Wq_self"], f32)
    bq = np.asarray(inputs["bq_self"], f32)
    Wk = np.asarray(inputs["Wk_self"], f32)
    Wv = np.asarray(inputs["Wv_self"], f32)
    bv = np.asarray(inputs["bv_self"], f32)
    Wo = np.asarray(inputs["Wo_self"], f32)
    bo = np.asarray(inputs["bo_self"], f32)
    Wqm = np.asarray(inputs["Wq_mem"], f32)
    bqm = np.asarray(inputs["bq_mem"], f32)
    Wom = np.asarray(inputs["Wo_mem"], f32)
    bom = np.asarray(inputs["bo_mem"], f32)
    W1 = np.asarray(inputs["W1"], f32)
    b1 = np.asarray(inputs["b1"], f32)
    W2 = np.asarray(inputs["W2"], f32)
    b2 = np.asarray(inputs["b2"], f32)
    tpad = np.asarray(inputs["tgt_padding_mask"], f32)[:, :, 0]  # [B, T]
    spad = np.asarray(inputs["src_padding_mask"], f32)[:, :, 0]  # [B, S]

    # host-folded weights
    A = np.einsum("hde,hfe->hdf", Wq, Wk)               # scores = x A x^T
    Wvo = np.einsum("hde,hef->hdf", Wv, Wo.reshape(H, D, D))
    bo_fold = bo + sum(bv[h] @ Wo[h * D:(h + 1) * D] for h in range(H))
    w_r = np.einsum("hde,he->hd", Wk, bq)               # r_self = x . w_r
    r_self = np.einsum("bsd,hd->bhs", x, w_r)           # [B, H, S]
    r_mem = np.einsum("bse,he->bhs", memk, bqm)         # [B, H, S]

    def colT(a2d, nd):  # [X, Y] with X = nd*P -> [P, nd, Y]
        return np.ascontiguousarray(
            a2d.reshape(nd, P, a2d.shape[1]).transpose(1, 0, 2))

    # weight layouts pre-shuffled so each per-head DMA is one contiguous
    # run per partition
    a8_q = q8(A, CSA)                                     # [H, D, D]
    a8_l = np.ascontiguousarray(                          # [P, H, ND, ND, P]
        a8_q.reshape(H, ND, P, ND, P).transpose(2, 0, 1, 3, 4))
    wqm_q = q8(Wqm, CSQ)
    wqm_l = np.ascontiguousarray(
        wqm_q.reshape(H, ND, P, ND, P).transpose(2, 0, 1, 3, 4))
    wvo_b = Wvo.astype(bf)                                # [H, D, D]
    wvo_l = np.ascontiguousarray(                         # [P, H, ND, D]
        wvo_b.reshape(H, ND, P, D).transpose(2, 0, 1, 3))
    wom_q = q8(Wom.reshape(H, D, D), CSOM)
    wom_l = np.ascontiguousarray(
        wom_q.reshape(H, ND, P, D).transpose(2, 0, 1, 3))
    w18_q = q8(W1, CSW1)                                  # [D, F]
    w18_l = np.ascontiguousarray(                         # [P, ND, NF, P]
        w18_q.reshape(ND, P, NF, P).transpose(1, 0, 2, 3))
    w28_q = q8(W2, CSW2)                                  # [F, D]
    w28_l = np.ascontiguousarray(                         # [P, NF, D]
        w28_q.reshape(NF, P, D).transpose(1, 0, 2))

    shared = {
        "a8": a8_l,
        "wvo": wvo_l,
        "wqm8": wqm_l,
        "wom8": wom_l,
        "w18": w18_l,
        "w28": w28_l,
        "b1_c": np.ascontiguousarray((b1 * F1S).reshape(NF, P).T).astype(f32),
        "b2_row": (b2 * F1S * CSW2).reshape(1, D).astype(bf),
        "bom_row": bom.reshape(1, D).astype(bf),
        "diag": np.ascontiguousarray(
            np.asarray(inputs["tgt_subsq_mask"], f32)[:P, :P].T),
    }

    in_maps = []
    for b in range(B):
        m = dict(shared)
        m["xT8"] = colT(q8(x[b].T, SX).reshape(D, T), ND)
        m["xTb"] = colT(x[b].T.astype(bf), ND)
        m["x32"] = np.ascontiguousarray(
            (x[b] + bo_fold[None, :]).reshape(NT, P, D).transpose(1, 0, 2))
        m["mkT8"] = colT(q8(memk[b].T, SX).reshape(D, S), ND)
        m["mv8"] = colT(q8(memv[b].T, SX).reshape(D, S), ND)
        # exp bias columns: [P, H*NS]; col h*NS+j, partition p -> s = j*P+p
        bs = (SCALE * r_self[b] + tpad[b][None, :] + np.log(ES_SELF))
        m["bias_self"] = np.ascontiguousarray(
            bs.reshape(H, NS, P).transpose(2, 0, 1).reshape(P, H * NS)
        ).astype(f32)
        bc = (SCALE * r_mem[b] + spad[b][None, :] + np.log(ES_CROSS))
        m["bias_cross"] = np.ascontiguousarray(
            bc.reshape(H, NS, P).transpose(2, 0, 1).reshape(P, H * NS)
        ).astype(f32)
        in_maps.append(m)
    return in_maps


def kernel(**inputs):
    from concourse.bass_utils import run_bass_kernel_spmd

    nc = _build()
    in_maps = make_in_maps(inputs)
    res = run_bass_kernel_spmd(nc, in_maps, list(range(B)))
    out = np.stack([np.asarray(res.results[i]["out"]) for i in range(B)])
    return out.astype(np.float32)


# revision 49
# speedup vs baseline: 1.0070x; 1.0070x over previous
"""Trainium2 Bass kernel for nn_DecoderLayer_19816979104174.

Data-parallel over batch: each of the 8 NeuronCores runs one batch element's
full decoder layer. v3 design:
  - Host-folded weights: A_h = Wq_h @ Wk_h^T collapses the Q and K projections
    into one (scores = x A x^T); Wvo_h = Wv_h @ Wo_h collapses the V projection
    and the output projection (bf16); Wom_h (cross) is applied to mem_values in
    fp8 DoubleRow. Bias terms that are not softmax-invariant fold into the
    per-s exp bias column (r_self / r_mem, host-computed).
  - fp8 e4m3 DoubleRow matmuls (2x PE rate) for the score path, the
    attention-weighted sums, the cross value fold, and the FFN; bf16 for the
    self value fold only (accuracy).
  - exp writes fp8 directly; rowsums via 128-wide fp8 ones matmuls (result
    lands pre-broadcast in PSUM); softmax reciprocals and LN rsqrt run on the
    scalar engine; attention weights are normalized in place so the AV matmuls
    accumulate all 8 heads directly in PSUM in [t, d] layout, consumed by
    LayerNorm straight out of PSUM.
  - x / memk / memv arrive pre-transposed (and pre-quantized) from the host;
    weights arrive pre-shuffled into partition-contiguous layout; input DMAs
    are split across two hardware queues (sync: x + weight streams,
    scalar/Activation: mem + residual tensors).
"""

import sys

sys.path.insert(0, "/opt/trn_rl_repo")
sys.path.insert(0, "/root/.axon_site/_ro/trn_rl_repo")

import numpy as np

B, T, S, D, H, F = 8, 1024, 1024, 512, 8, 2048
P = 128
ND, NS, NT, NF, NC2 = D // P, S // P, T // P, F // P, T // 512
SCALE = 1.0 / float(np.sqrt(D))
LN_EPS = 1e-5

# fp8 scale constants (compile-time)
SX = 16.0       # x, x1, x2, memk, memv quantization scale
SXA = 64.0      # xA (score-path projection) fp8 scale
SQM = 32.0      # qm fp8 scale
SV = 64.0       # v' / mv fp8 scale
SNORM = 128.0   # normalized attention weights are stored as a*SNORM
F1S = 64.0      # f1 (relu output) fp8 scale
CSA = 4096.0    # host scale on A_h
CSQ = 1536.0    # host scale on Wqm_h
CSOM = 1536.0   # host scale on Wo_mem (fp8)
CSW1 = 1536.0   # host scale on W1
CSW2 = 1536.0   # host scale on W2
ES_SELF = 32.0  # exp output scale, self attention
ES_CROSS = 8.0  # exp output scale, cross attention

_CACHE = {}


def _build():
    if "nc" in _CACHE:
        return _CACHE["nc"]

    import concourse.tile as tile
    import concourse.mybir as mybir
    from concourse import bacc
    from concourse.masks import make_identity
    from contextlib import ExitStack

    bf16 = mybir.dt.bfloat16
    f32 = mybir.dt.float32
    fp8 = mybir.dt.float8e4
    AF = mybir.ActivationFunctionType
    OP = mybir.AluOpType
    DR = mybir.MatmulPerfMode.DoubleRow

    nc = bacc.Bacc("TRN2")

    # ---- DRAM I/O -----------------------------------------------------
    d_xT8 = nc.dram_tensor("xT8", [P, ND, T], fp8, kind="ExternalInput")
    d_xTb = nc.dram_tensor("xTb", [P, ND, T], bf16, kind="ExternalInput")
    d_x32 = nc.dram_tensor("x32", [P, NT, D], f32, kind="ExternalInput")
    d_mkT8 = nc.dram_tensor("mkT8", [P, ND, S], fp8, kind="ExternalInput")
    d_mv8 = nc.dram_tensor("mv8", [P, ND, S], fp8, kind="ExternalInput")
    d_a8 = nc.dram_tensor("a8", [P, H, ND, ND, P], fp8, kind="ExternalInput")
    d_wvo = nc.dram_tensor("wvo", [P, H, ND, D], bf16, kind="ExternalInput")
    d_wqm8 = nc.dram_tensor("wqm8", [P, H, ND, ND, P], fp8,
                            kind="ExternalInput")
    d_wom8 = nc.dram_tensor("wom8", [P, H, ND, D], fp8, kind="ExternalInput")
    d_w18 = nc.dram_tensor("w18", [P, ND, NF, P], fp8, kind="ExternalInput")
    d_w28 = nc.dram_tensor("w28", [P, NF, D], fp8, kind="ExternalInput")
    d_bs = nc.dram_tensor("bias_self", [P, H * NS], f32, kind="ExternalInput")
    d_bc = nc.dram_tensor("bias_cross", [P, H * NS], f32, kind="ExternalInput")
    d_b1 = nc.dram_tensor("b1_c", [P, NF], f32, kind="ExternalInput")
    d_b2 = nc.dram_tensor("b2_row", [1, D], bf16, kind="ExternalInput")
    d_bom = nc.dram_tensor("bom_row", [1, D], bf16, kind="ExternalInput")
    d_diag = nc.dram_tensor("diag", [P, P], f32, kind="ExternalInput")
    d_out = nc.dram_tensor("out", [T, D], f32, kind="ExternalOutput")

    with tile.TileContext(nc) as tc, ExitStack() as ctx:
        const = ctx.enter_context(tc.tile_pool(name="const", bufs=1))
        small = ctx.enter_context(tc.tile_pool(name="small", bufs=2))
        psum_mm = ctx.enter_context(tc.tile_pool(name="psum_mm", bufs=3, space="PSUM"))
        psum_av = ctx.enter_context(tc.tile_pool(name="psum_av", bufs=2, space="PSUM"))
        psum_rs = ctx.enter_context(tc.tile_pool(name="psum_rs", bufs=2, space="PSUM"))
        psum_tr = ctx.enter_context(tc.tile_pool(name="psum_tr", bufs=1, space="PSUM"))

        # ---- pool opens (lifetime order; DMAs issued below) -----------
        es_big = ExitStack()
        big = es_big.enter_context(tc.tile_pool(name="big", bufs=1))
        expTn = [big.tile([P, NS, T], fp8, tag=f"expTn{h}", name=f"expTn{h}")
                 for h in range(H)]
        v8 = [big.tile([P, NS, D], fp8, tag=f"v8_{h}", name=f"v8_{h}")
              for h in range(H)]

        es_mem = ExitStack()
        mem_pool = es_mem.enter_context(tc.tile_pool(name="mem", bufs=1))
        x1_sb = mem_pool.tile([P, NT, D], f32, tag="x1")
        x1T8 = mem_pool.tile([P, ND, T], fp8, tag="x1T8")
        # per-(head, t-block) softmax inverse row sums, [t_part, 1] layout
        inv1_sb = mem_pool.tile([P, H * NT], f32, tag="inv1")
        inv2_sb = mem_pool.tile([P, H * NT], f32, tag="inv2")
        bom_bc = mem_pool.tile([P, D], f32, tag="bom_bc")

        # mem/residual tensors (scalar hwdge queue) live only through
        # phase 2/3; their pool closes with ws2_stack so the FFN pool can
        # reuse the space
        ws2_stack = ExitStack()
        mem2_pool = ws2_stack.enter_context(tc.tile_pool(name="mem2", bufs=1))
        mkT8 = mem2_pool.tile([P, ND, S], fp8)
        mv8 = mem2_pool.tile([P, ND, S], fp8)
        x32_sb = mem2_pool.tile([P, NT, D], f32)
        wstream2 = ws2_stack.enter_context(tc.tile_pool(name="wstream2", bufs=2))

        es_x = ExitStack()
        x_pool = es_x.enter_context(tc.tile_pool(name="xp", bufs=1))
        ws_stack = ExitStack()
        wstream = ws_stack.enter_context(tc.tile_pool(name="wstream", bufs=2))

        # ---- x + phase-1 weight stream: sync queue issued FIRST so the
        # PE's first matmul inputs land as early as possible ------------
        xT8 = x_pool.tile([P, ND, T], fp8)
        nc.sync.dma_start(out=xT8[:], in_=d_xT8.ap())
        a_tiles = []
        wvo_tiles = []
        xTb = None
        for h in range(H):
            a_t = wstream.tile([P, ND, ND, P], fp8, tag="a8")
            nc.sync.dma_start(out=a_t[:], in_=d_a8.ap()[:, h])
            a_tiles.append(a_t)
            if h == 0:
                xTb = x_pool.tile([P, ND, T], bf16)
                nc.sync.dma_start(out=xTb[:], in_=d_xTb.ap())
            wvo_t = wstream.tile([P, ND, D], bf16, tag="wvo")
            nc.sync.dma_start(out=wvo_t[:], in_=d_wvo.ap()[:, h])
            wvo_tiles.append(wvo_t)

        # ---- constants (gpsimd queue) ---------------------------------
        ident_f = const.tile([P, P], f32)
        make_identity(nc, ident_f)
        ones8 = const.tile([P, 2, P], fp8)
        nc.vector.memset(ones8[:], 1.0 / SNORM)
        ones_row = const.tile([1, P], bf16)
        nc.vector.memset(ones_row[:], 1.0)
        eps_t = const.tile([P, 1], f32)
        nc.vector.memset(eps_t[:], LN_EPS)
        e0c = const.tile([P, 1], bf16)
        nc.vector.memset(e0c[:], 0.0)
        nc.vector.memset(e0c[0:1, :], SNORM * SV)
        diag_sb = const.tile([P, P], f32)
        nc.gpsimd.dma_start(out=diag_sb[:], in_=d_diag.ap())
        bs_sb = const.tile([P, H * NS], f32)
        nc.gpsimd.dma_start(out=bs_sb[:], in_=d_bs.ap())
        bc_sb = const.tile([P, H * NS], f32)
        nc.gpsimd.dma_start(out=bc_sb[:], in_=d_bc.ap())
        b1_sb = const.tile([P, NF], f32)
        nc.gpsimd.dma_start(out=b1_sb[:], in_=d_b1.ap())
        b2_sb = const.tile([1, D], bf16)
        nc.gpsimd.dma_start(out=b2_sb[:], in_=d_b2.ap())
        bom_sb = const.tile([1, D], bf16)
        nc.gpsimd.dma_start(out=bom_sb[:], in_=d_bom.ap())

        # big-tensor zero fills (gpsimd)
        for h in range(H):
            nc.gpsimd.memset(expTn[h][:], 0.0)

        # mem tensors + residual x: scalar (Activation) hwdge queue
        nc.scalar.dma_start(out=mkT8[:], in_=d_mkT8.ap())
        nc.scalar.dma_start(out=mv8[:], in_=d_mv8.ap())
        nc.scalar.dma_start(out=x32_sb[:], in_=d_x32.ap())

        # ---- helpers ---------------------------------------------------
        def ln_psum(ps_ap, inv, resid_ap, out_ap):
            """out = LN(ps*inv + resid)   (gamma=1, beta=0)"""
            res = small.tile([P, D], f32, tag="ln_res")
            nc.vector.scalar_tensor_tensor(
                out=res[:], in0=ps_ap, scalar=inv, in1=resid_ap,
                op0=OP.mult, op1=OP.add)
            stats = small.tile([P, 6], f32, tag="ln_stats")
            nc.vector.bn_stats(stats[:], res[:])
            mv2 = small.tile([P, 2], f32, tag="ln_mv")
            nc.vector.bn_aggr(mv2[:], stats[:])
            std = small.tile([P, 1], f32, tag="ln_std")
            nc.scalar.activation(std[:], mv2[:, 1:2], AF.Sqrt, bias=eps_t[:])
            istd = small.tile([P, 1], f32, tag="ln_istd")
            nc.vector.reciprocal(istd[:], std[:])
            nc.vector.tensor_scalar(
                out=out_ap, in0=res[:], scalar1=mv2[:, 0:1], scalar2=istd[:],
                op0=OP.subtract, op1=OP.mult)

        def ln_sbuf(acc, out_ap):
            """out = LN(acc)   (gamma=1, beta=0); acc is SBUF f32 [P, D]."""
            stats = small.tile([P, 6], f32, tag="ln_stats")
            nc.vector.bn_stats(stats[:], acc[:])
            mv2 = small.tile([P, 2], f32, tag="ln_mv")
            nc.vector.bn_aggr(mv2[:], stats[:])
            std = small.tile([P, 1], f32, tag="ln_std")
            nc.scalar.activation(std[:], mv2[:, 1:2], AF.Sqrt, bias=eps_t[:])
            istd = small.tile([P, 1], f32, tag="ln_istd")
            nc.vector.reciprocal(istd[:], std[:])
            nc.vector.tensor_scalar(
                out=out_ap, in0=acc[:], scalar1=mv2[:, 0:1], scalar2=istd[:],
                op0=OP.subtract, op1=OP.mult)

        def attn_head(h, qsrcT, kT, bias_sb, causal, exp_scale, rsb_pool,
                      inv_sb, filler=None):
            """scores -> exp(fp8, unnormalized) -> rowsum -> inv[t,1], head h.

            qsrcT: fp8 [P, ND, T] query-side (already scaled); kT: fp8 key-side
            [P, ND, S]. Writes raw exp weights into expTn[h] and per-t-block
            reciprocal row sums (x 1/(SNORM*SV)) into inv_sb[:, h*NT + tb].
            filler(c) emits independent PE work between the score matmuls and
            the rowsum (which must wait for the scalar-engine exps).
            """
            for c in range(NC2):
                jmax = 4 * (c + 1) if causal else NS
                for j in range(jmax):
                    lo = max(j * P, c * 512) if causal else c * 512
                    w = (c + 1) * 512 - lo
                    ps = psum_mm.tile([P, 512], f32, tag="mm")
                    for ep in range(2):
                        nc.tensor.matmul(
                            ps[:, :w],
                            lhsT=kT[:, 2 * ep:2 * ep + 2, j * P:(j + 1) * P],
                            rhs=qsrcT[:, 2 * ep:2 * ep + 2, lo:(c + 1) * 512],
                            start=(ep == 0), stop=(ep == 1), perf_mode=DR)
                    if causal and lo == j * P:
                        nc.vector.tensor_tensor(
                            out=ps[:, 0:P], in0=ps[:, 0:P], in1=diag_sb[:],
                            op=OP.add)
                    nc.scalar.activation(
                        expTn[h][:, j, lo:(c + 1) * 512], ps[:, :w], AF.Exp,
                        bias=bias_sb[:, h * NS + j:h * NS + j + 1],
                        scale=exp_scale)
                if filler is not None:
                    filler(c)
                rs = psum_rs.tile([P, 512], f32, tag="rs")
                for jp in range(jmax // 2):
                    nc.tensor.matmul(
                        rs[:], lhsT=ones8[:],
                        rhs=expTn[h][:, 2 * jp:2 * jp + 2, c * 512:(c + 1) * 512],
                        start=(jp == 0), stop=(jp == jmax // 2 - 1), perf_mode=DR)
                # transpose the (partition-broadcast) row sums into [t, 1]
                # via one-hot matmuls, then a tiny reciprocal
                rs_sb = rsb_pool.tile([P, 512], bf16, tag="rssb")
                nc.vector.tensor_scalar_mul(rs_sb[:], rs[:], 1.0)
                for tbi in range(4):
                    nc.tensor.matmul(
                        rs[:, 508 + tbi:509 + tbi],
                        lhsT=rs_sb[:, tbi * P:(tbi + 1) * P], rhs=e0c[:],
                        start=True, stop=True)
                nc.vector.reciprocal_approx_fast(
                    out=inv_sb[:, h * NT + 4 * c:h * NT + 4 * c + 4],
                    in_=rs[:, 508:512])

        # ============ phase 1: self attention scores/weights ============
        with tc.tile_pool(name="xap", bufs=2) as xa_pool, \
             tc.tile_pool(name="rbp", bufs=2) as rsb_pool:
            for h in range(H):
                a_t = a_tiles[h]
                wvo_t = wvo_tiles[h]
                xAT = xa_pool.tile([P, ND, T], fp8, tag="xAT")
                for eb in range(ND):
                    for c in range(NC2):
                        ps = psum_mm.tile([P, 512], f32, tag="mm")
                        for kp in range(2):
                            nc.tensor.matmul(
                                ps[:], lhsT=a_t[:, 2 * kp:2 * kp + 2, eb, :],
                                rhs=xT8[:, 2 * kp:2 * kp + 2, c * 512:(c + 1) * 512],
                                start=(kp == 0), stop=(kp == 1), perf_mode=DR)
                        nc.scalar.activation(
                            xAT[:, eb, c * 512:(c + 1) * 512], ps[:], AF.Copy,
                            scale=SXA / (CSA * SX))

                def vo_fill(c, wvo_t=wvo_t, hh=h):
                    for sb_ in range(4 * c, 4 * c + 4):
                        ps = psum_mm.tile([P, 512], f32, tag="mm")
                        for kt in range(ND):
                            nc.tensor.matmul(
                                ps[:], lhsT=xTb[:, kt, sb_ * P:(sb_ + 1) * P],
                                rhs=wvo_t[:, kt, :],
                                start=(kt == 0), stop=(kt == ND - 1))
                        nc.vector.tensor_scalar_mul(
                            v8[hh][:, sb_, :], ps[:], SV)

                attn_head(h, xAT, xT8, bs_sb, True, SCALE / (SX * SXA),
                          rsb_pool, inv1_sb, filler=vo_fill)
        ws_stack.close()
        es_x.close()

        # phase-3 weight stream: issue all DMAs now (sync queue) so they
        # land during phase 2 (double-buffering emerges from tile WAR deps)
        wqm_tiles = []
        wom_tiles = []
        for h in range(H):
            wqm_t = wstream2.tile([P, ND, ND, P], fp8, tag="wqm")
            nc.sync.dma_start(out=wqm_t[:], in_=d_wqm8.ap()[:, h])
            wqm_tiles.append(wqm_t)
            wom_t = wstream2.tile([P, ND, D], fp8, tag="wom")
            nc.sync.dma_start(out=wom_t[:], in_=d_wom8.ap()[:, h])
            wom_tiles.append(wom_t)

        # ===== phase 2: self AV (per-head PSUM + inv-scaled combine) ====
        for tb in range(NT):
            acc = small.tile([P, D], f32, tag="acc")
            npair = (tb + 2) // 2
            for h in range(H):
                ps_av = psum_av.tile([P, 512], f32, tag="av")
                for jp in range(npair):
                    nc.tensor.matmul(
                        ps_av[:],
                        lhsT=expTn[h][:, 2 * jp:2 * jp + 2, tb * P:(tb + 1) * P],
                        rhs=v8[h][:, 2 * jp:2 * jp + 2, :],
                        start=(jp == 0), stop=(jp == npair - 1),
                        perf_mode=DR)
                nc.vector.scalar_tensor_tensor(
                    out=acc[:], in0=ps_av[:],
                    scalar=inv1_sb[:, h * NT + tb:h * NT + tb + 1],
                    in1=(x32_sb[:, tb, :] if h == 0 else acc[:]),
                    op0=OP.mult, op1=OP.add)
            ln_sbuf(acc, x1_sb[:, tb, :])
            tr = psum_tr.tile([P, 512], f32, tag="tr")
            for dt in range(ND):
                nc.tensor.transpose(
                    tr[:, dt * P:(dt + 1) * P],
                    x1_sb[:, tb, dt * P:(dt + 1) * P], ident_f[:])
            nc.scalar.activation(
                x1T8[:, :, tb * P:(tb + 1) * P],
                tr[:].rearrange("p (d t) -> p d t", d=ND), AF.Copy, scale=SX)

        # ============ phase 3: cross attention scores/weights ===========
        with tc.tile_pool(name="qmp", bufs=2) as qm_pool, \
             tc.tile_pool(name="rbp2", bufs=2) as rsb_pool2:
            for h in range(H):
                wqm_t = wqm_tiles[h]
                wom_t = wom_tiles[h]
                qmT = qm_pool.tile([P, ND, T], fp8, tag="qmT")
                for eb in range(ND):
                    for c in range(NC2):
                        ps = psum_mm.tile([P, 512], f32, tag="mm")
                        for kp in range(2):
                            nc.tensor.matmul(
                                ps[:], lhsT=wqm_t[:, 2 * kp:2 * kp + 2, eb, :],
                                rhs=x1T8[:, 2 * kp:2 * kp + 2, c * 512:(c + 1) * 512],
                                start=(kp == 0), stop=(kp == 1), perf_mode=DR)
                        nc.scalar.activation(
                            qmT[:, eb, c * 512:(c + 1) * 512], ps[:], AF.Copy,
                            scale=SQM / (CSQ * SX))

                def om_fill(c, wom_t=wom_t, hh=h):
                    for sb_ in range(4 * c, 4 * c + 4):
                        ps = psum_mm.tile([P, 512], f32, tag="mm")
                        for kp in range(2):
                            nc.tensor.matmul(
                                ps[:],
                                lhsT=mv8[:, 2 * kp:2 * kp + 2, sb_ * P:(sb_ + 1) * P],
                                rhs=wom_t[:, 2 * kp:2 * kp + 2, :],
                                start=(kp == 0), stop=(kp == 1), perf_mode=DR)
                        nc.vector.tensor_scalar_mul(
                            v8[hh][:, sb_, :], ps[:], SV / (SX * CSOM))

                attn_head(h, qmT, mkT8, bc_sb, False, SCALE / (SX * SQM),
                          rsb_pool2, inv2_sb, filler=om_fill)
        ws2_stack.close()

        es_x2 = ExitStack()
        x2_pool = es_x2.enter_context(tc.tile_pool(name="x2p", bufs=1))
        x2_sb = x2_pool.tile([P, NT, D], f32, tag="x2")
        x2T8 = x2_pool.tile([P, ND, T], fp8, tag="x2T8")

        # FFN weights: DMA now so they land during cross AV
        es_ffn = ExitStack()
        ffn_pool = es_ffn.enter_context(tc.tile_pool(name="ffn", bufs=1))
        w18_t = ffn_pool.tile([P, ND, NF, P], fp8, tag="w18")
        nc.sync.dma_start(out=w18_t[:], in_=d_w18.ap())
        w28_t = ffn_pool.tile([P, NF, D], fp8, tag="w28")
        nc.sync.dma_start(out=w28_t[:], in_=d_w28.ap())
        f1T8 = ffn_pool.tile([P, NF, T], fp8, tag="f1T8")

        def f1_chunk(c):
            for fb in range(NF):
                ps = psum_mm.tile([P, 512], f32, tag="mm")
                for kp in range(2):
                    nc.tensor.matmul(
                        ps[:], lhsT=w18_t[:, 2 * kp:2 * kp + 2, fb, :],
                        rhs=x2T8[:, 2 * kp:2 * kp + 2, c * 512:(c + 1) * 512],
                        start=(kp == 0), stop=(kp == 1), perf_mode=DR)
                nc.scalar.activation(
                    f1T8[:, fb, c * 512:(c + 1) * 512], ps[:], AF.Relu,
                    bias=b1_sb[:, fb:fb + 1], scale=F1S / (SX * CSW1))

        # materialize the bom bias row broadcast across partitions (once)
        ps_bb = psum_av.tile([P, 512], f32, tag="av")
        nc.tensor.matmul(
            ps_bb[:], lhsT=ones_row[:, 0:P], rhs=bom_sb[:],
            start=True, stop=True)
        nc.vector.tensor_scalar_mul(bom_bc[:], ps_bb[:], 1.0)

        # ===== phase 4: cross AV (per-head PSUM + combine) + LN2 + FFN ==
        for tb in range(NT):
            acc = small.tile([P, D], f32, tag="acc")
            for h in range(H):
                ps_av = psum_av.tile([P, 512], f32, tag="av")
                for jp in range(NS // 2):
                    nc.tensor.matmul(
                        ps_av[:],
                        lhsT=expTn[h][:, 2 * jp:2 * jp + 2, tb * P:(tb + 1) * P],
                        rhs=v8[h][:, 2 * jp:2 * jp + 2, :],
                        start=(jp == 0), stop=(jp == NS // 2 - 1),
                        perf_mode=DR)
                nc.vector.scalar_tensor_tensor(
                    out=acc[:], in0=ps_av[:],
                    scalar=inv2_sb[:, h * NT + tb:h * NT + tb + 1],
                    in1=(x1_sb[:, tb, :] if h == 0 else acc[:]),
                    op0=OP.mult, op1=OP.add)
            nc.gpsimd.tensor_tensor(
                out=acc[:], in0=acc[:], in1=bom_bc[:], op=OP.add)
            ln_sbuf(acc, x2_sb[:, tb, :])
            if tb < 4:
                tr = psum_tr.tile([P, 512], f32, tag="tr")
                for dt in range(ND):
                    nc.tensor.transpose(
                        tr[:, dt * P:(dt + 1) * P],
                        x2_sb[:, tb, dt * P:(dt + 1) * P], ident_f[:])
                nc.scalar.activation(
                    x2T8[:, :, tb * P:(tb + 1) * P],
                    tr[:].rearrange("p (d t) -> p d t", d=ND), AF.Copy,
                    scale=SX)
            if tb == 3:
                f1_chunk(0)

        def f2_block(tb):
            ps = psum_mm.tile([P, 512], f32, tag="mm")
            nc.tensor.matmul(
                ps[:], lhsT=ones_row[:, 0:P], rhs=b2_sb[:],
                start=True, stop=False)
            for kp in range(NF // 2):
                nc.tensor.matmul(
                    ps[:], lhsT=f1T8[:, 2 * kp:2 * kp + 2, tb * P:(tb + 1) * P],
                    rhs=w28_t[:, 2 * kp:2 * kp + 2, :],
                    start=False, stop=(kp == NF // 2 - 1), perf_mode=DR)
            out_sb = small.tile([P, D], f32, tag="out_sb")
            ln_psum(ps[:], 1.0 / (F1S * CSW2), x2_sb[:, tb, :], out_sb[:])
            nc.sync.dma_start(
                out=d_out.ap().rearrange("(tb p) d -> p tb d", p=P)[:, tb, :],
                in_=out_sb[:])

        # f2 for t-blocks 0-3 only needs f1 chunk 0.  The chunk-1
        # transposes (which wait on the tb4-7 LayerNorm chains) interleave
        # with those f2 blocks so the PE always has independent work.
        for tb in range(4):
            tr = psum_tr.tile([P, 512], f32, tag="tr")
            for dt in range(ND):
                nc.tensor.transpose(
                    tr[:, dt * P:(dt + 1) * P],
                    x2_sb[:, tb + 4, dt * P:(dt + 1) * P], ident_f[:])
            nc.scalar.activation(
                x2T8[:, :, (tb + 4) * P:(tb + 5) * P],
                tr[:].rearrange("p (d t) -> p d t", d=ND), AF.Copy, scale=SX)
            f2_block(tb)
        f1_chunk(1)
        for tb in range(4, NT):
            f2_block(tb)
        es_ffn.close()
        es_x2.close()
        es_mem.close()
        es_big.close()

    nc.compile()
    _CACHE["nc"] = nc
    return nc


def make_in_maps(inputs):
    import ml_dtypes

    bf = ml_dtypes.bfloat16
    e4 = ml_dtypes.float8_e4m3
    f32 = np.float32

    def q8(a, s):
        return np.clip(np.asarray(a, f32) * s, -240.0, 240.0).astype(e4)

    x = np.asarray(inputs["x"], f32)
    memk = np.asarray(inputs["mem_keys"], f32)
    memv = np.asarray(inputs["mem_values"], f32)
    Wq = np.asarray(inputs["Wq_self"], f32)
    bq = np.asarray(inputs["bq_self"], f32)
    Wk = np.asarray(inputs["Wk_self"], f32)
    Wv = np.asarray(inputs["Wv_self"], f32)
    bv = np.asarray(inputs["bv_self"], f32)
    Wo = np.asarray(inputs["Wo_self"], f32)
    bo = np.asarray(inputs["bo_self"], f32)
    Wqm = np.asarray(inputs["Wq_mem"], f32)
    bqm = np.asarray(inputs["bq_mem"], f32)
    Wom = np.asarray(inputs["Wo_mem"], f32)
    bom = np.asarray(inputs["bo_mem"], f32)
    W1 = np.asarray(inputs["W1"], f32)
    b1 = np.asarray(inputs["b1"], f32)
    W2 = np.asarray(inputs["W2"], f32)
    b2 = np.asarray(inputs["b2"], f32)
    tpad = np.asarray(inputs["tgt_padding_mask"], f32)[:, :, 0]  # [B, T]
    spad = np.asarray(inputs["src_padding_mask"], f32)[:, :, 0]  # [B, S]

    # host-folded weights
    A = np.einsum("hde,hfe->hdf", Wq, Wk)               # scores = x A x^T
    Wvo = np.einsum("hde,hef->hdf", Wv, Wo.reshape(H, D, D))
    bo_fold = bo + sum(bv[h] @ Wo[h * D:(h + 1) * D] for h in range(H))
    w_r = np.einsum("hde,he->hd", Wk, bq)               # r_self = x . w_r
    r_self = np.einsum("bsd,hd->bhs", x, w_r)           # [B, H, S]
    r_mem = np.einsum("bse,he->bhs", memk, bqm)         # [B, H, S]

    def colT(a2d, nd):  # [X, Y] with X = nd*P -> [P, nd, Y]
        return np.ascontiguousarray(
            a2d.reshape(nd, P, a2d.shape[1]).transpose(1, 0, 2))

    # weight layouts pre-shuffled so each per-head DMA is one contiguous
    # run per partition
    a8_q = q8(A, CSA)                                     # [H, D, D]
    a8_l = np.ascontiguousarray(                          # [P, H, ND, ND, P]
        a8_q.reshape(H, ND, P, ND, P).transpose(2, 0, 1, 3, 4))
    wqm_q = q8(Wqm, CSQ)
    wqm_l = np.ascontiguousarray(
        wqm_q.reshape(H, ND, P, ND, P).transpose(2, 0, 1, 3, 4))
    wvo_b = Wvo.astype(bf)                                # [H, D, D]
    wvo_l = np.ascontiguousarray(                         # [P, H, ND, D]
        wvo_b.reshape(H, ND, P, D).transpose(2, 0, 1, 3))
    wom_q = q8(Wom.reshape(H, D, D), CSOM)
    wom_l = np.ascontiguousarray(
        wom_q.reshape(H, ND, P, D).transpose(2, 0, 1, 3))
    w18_q = q8(W1, CSW1)                                  # [D, F]
    w18_l = np.ascontiguousarray(                         # [P, ND, NF, P]
        w18_q.reshape(ND, P, NF, P).transpose(1, 0, 2, 3))
    w28_q = q8(W2, CSW2)                                  # [F, D]
    w28_l = np.ascontiguousarray(                         # [P, NF, D]
        w28_q.reshape(NF, P, D).transpose(1, 0, 2))

    shared = {
        "a8": a8_l,
        "wvo": wvo_l,
        "wqm8": wqm_l,
        "wom8": wom_l,
        "w18": w18_l,
        "w28": w28_l,
        "b1_c": np.ascontiguousarray((b1 * F1S).reshape(NF, P).T).astype(f32),
        "b2_row": (b2 * F1S * CSW2).reshape(1, D).astype(bf),
        "bom_row": bom.reshape(1, D).astype(bf),
        "diag": np.ascontiguousarray(
            np.asarray(inputs["tgt_subsq_mask"], f32)[:P, :P].T),
    }

    in_maps = []
    for b in range(B):
        m = dict(shared)
        m["xT8"] = colT(q8(x[b].T, SX).reshape(D, T), ND)
        m["xTb"] = colT(x[b].T.astype(bf), ND)
        m["x32"] = np.ascontiguousarray(
            (x[b] + bo_fold[None, :]).reshape(NT, P, D).transpose(1, 0, 2))
        m["mkT8"] = colT(q8(memk[b].T, SX).reshape(D, S), ND)
        m["mv8"] = colT(q8(memv[b].T, SX).reshape(D, S), ND)
        # exp bias columns: [P, H*NS]; col h*NS+j, partition p -> s = j*P+p
        bs = (SCALE * r_self[b] + tpad[b][None, :] + np.log(ES_SELF))
        m["bias_self"] = np.ascontiguousarray(
            bs.reshape(H, NS, P).transpose(2, 0, 1).reshape(P, H * NS)
        ).astype(f32)
        bc = (SCALE * r_mem[b] + spad[b][None, :] + np.log(ES_CROSS))
        m["bias_cross"] = np.ascontiguousarray(
            bc.reshape(H, NS, P).transpose(2, 0, 1).reshape(P, H * NS)
        ).astype(f32)
        in_maps.append(m)
    return in_maps


def kernel(**inputs):
    from concourse.bass_utils import run_bass_kernel_spmd

    nc = _build()
    in_maps = make_in_maps(inputs)
    res = run_bass_kernel_spmd(nc, in_maps, list(range(B)))
    out = np.stack([np.asarray(res.results[i]["out"]) for i in range(B)])
    return out.astype(np.float32)
